# revision 32
# baseline (speedup 1.0000x reference)
"""Trainium2 Bass kernel for nn_MultiDirectionalSpatialScanner.

Bidirectional Mamba-style spatial scanner, B=32 H=W=32 D=384, d_state=4.
Sharding: data-parallel over batch across 8 cores (4 batches/core).

v2 design vs v1:
  - fp8e4 + DoubleRow matmuls for in_proj(conv-folded), z, gw, w2 with
    power-of-2 weight prescales folded into ACT eviction scales.
  - dt computed via one fused matmul (x_proj_dt @ dt_proj collapsed on host),
    softplus via the Square trick with per-channel bias.
  - B/C broadcast rows produced via a DRAM round-trip broadcast DMA instead
    of 32 PE matmuls + 32 ACT evictions.
  - states concatenated along free dim: one [128, 3*1024] scan per (dir, s)
    with es=0 at tile seams (resets the recurrence), wide elementwise ops.
  - LN applies and dtype casts on ACT (per-partition scale/bias); tables
    ordered so only ~4 ACT table switches occur per batch.
  - residual scaled by 64 (LN2 is scale-invariant) so w2's fp8 prescale
    needs no extra correction pass.
  - batched input/output/broadcast DMAs.
"""

import math
import numpy as np
from contextlib import ExitStack

import ml_dtypes
import concourse.bass as bass
import concourse.bacc as bacc
import concourse.tile as tile
from concourse.tile import add_dep_helper
from concourse import mybir
from concourse import bass_utils

F32 = mybir.dt.float32
BF16 = mybir.dt.bfloat16
FP8 = mybir.dt.float8e4
AF = mybir.ActivationFunctionType
OP = mybir.AluOpType
PM = mybir.MatmulPerfMode

B, Hh, Ww, D = 32, 32, 32, 384
L = Hh * Ww                 # 1024
ND, DST, DCONV, DIN, DTR = 2, 4, 3, 384, 24
NCORES = 8
BL = B // NCORES            # 4 batches per core
NDT = DIN // 128            # 3 feature tiles
NTT = L // 128              # 8 token tiles per batch
EPS = 1e-5
C_SP = 0.1931471806         # ln2 - 1/2 (softplus quad constant)
SQ_A = 0.3535533906         # 1/sqrt(8)
SQ_B = 0.7071067812         # 1/sqrt(2)
CSCALE = 16.0               # C-row prescale so yn fits fp8 nicely
RSCALE = 64.0               # residual/w2 prescale (LN2 scale-invariant)
BF = ml_dtypes.bfloat16
E4M3 = ml_dtypes.float8_e4m3


def _pow2_scale(w, target=240.0):
    """Power-of-two scale s so |w*s| <= target."""
    a = float(np.abs(w).max())
    if a == 0.0:
        return 1.0
    return 2.0 ** math.floor(math.log2(target / a))


def _pos_embed_np(H, W, Dm):
    ph = (np.arange(H, dtype=np.float32) / (H - 1)) * 2 - 1
    pw = (np.arange(W, dtype=np.float32) / (W - 1)) * 2 - 1
    gh, gw = np.meshgrid(ph, pw, indexing="ij")
    div = np.exp(np.arange(0, Dm, 2, dtype=np.float32) * (-math.log(10000.0) / Dm))
    d4 = div[::2]
    pe = np.zeros((H, W, Dm), np.float32)
    pe[:, :, 0::4] = np.sin(gh[..., None] * d4)
    pe[:, :, 1::4] = np.cos(gh[..., None] * d4)
    pe[:, :, 2::4] = np.sin(gw[..., None] * d4)
    pe[:, :, 3::4] = np.cos(gw[..., None] * d4)
    return pe.reshape(H * W, Dm)


def _host_weights(inp):
    g = np.asarray(inp["ln_in_g"], np.float32)
    bta = np.asarray(inp["ln_in_b"], np.float32)
    ipw = np.asarray(inp["in_proj_w"], np.float32)      # [2, D, 2*DIN]
    cw = np.asarray(inp["conv_w"], np.float32)          # [2, DIN, 3]
    xpw = np.asarray(inp["x_proj_w"], np.float32)       # [2, DIN, 32]
    dtw = np.asarray(inp["dt_proj_w"], np.float32)      # [2, 24, DIN]
    dtb = np.asarray(inp["dt_proj_b"], np.float32)      # [2, DIN]
    A_log = np.asarray(inp["A_log"], np.float32)        # [2, DIN, 4]
    Dp = np.asarray(inp["D_param"], np.float32)         # [2, DIN]
    opw = np.asarray(inp["out_proj_w"], np.float32)     # [2, DIN, D]
    dpw = np.asarray(inp["dir_proj_w"], np.float32)     # [2, D, D]
    fw1 = np.asarray(inp["fusion_w1"], np.float32)      # [2D, 2D]
    fw2 = np.asarray(inp["fusion_w2"], np.float32)      # [2D, D]
    dw = np.asarray(inp["dir_weights"], np.float32)     # [2]

    pe = _pos_embed_np(Hh, Ww, D)                       # [L, D]
    sig = (bta[None, :] + pe) / g[None, :]              # [L, D]

    # xi weights with LN gamma + conv tap folded, fp8 with per-dir scale.
    wxi = np.stack([g[:, None] * ipw[i][:, :DIN] for i in range(ND)])  # [2,D,DIN]
    wxik = np.zeros((ND, DCONV, 4, 128, DIN), np.float32)  # [dir,tap,kt(pad),128,m]
    for i in range(ND):
        for k in range(DCONV):
            wk = wxi[i] * cw[i][None, :, k]             # [D(K), DIN(M)]
            wxik[i, k, :NDT] = wk.reshape(NDT, 128, DIN)
    s_xi = [_pow2_scale(wxik[i]) for i in range(ND)]
    for i in range(ND):
        wxik[i] *= s_xi[i]
    # layout [128, dir, tap, kt, M]
    wxik_t = np.transpose(wxik, (3, 0, 1, 2, 4)).copy()

    wz = np.zeros((ND, 4, 128, DIN), np.float32)
    for i in range(ND):
        wz[i, :NDT] = (g[:, None] * ipw[i][:, DIN:]).reshape(NDT, 128, DIN)
    s_z = [_pow2_scale(wz[i]) for i in range(ND)]
    for i in range(ND):
        wz[i] *= s_z[i]
    wz_t = np.transpose(wz, (2, 0, 1, 3)).copy()        # [128, dir, kt, M]

    # fused dt projection: xcv @ (xpw_dt @ dtw) + dtb, softplus-quad bias
    wdtd = np.stack([xpw[i][:, :DTR] @ dtw[i] for i in range(ND)])  # [2, DIN, DIN]
    wdtd_t = wdtd.reshape(ND, NDT, 128, DIN).transpose(2, 0, 1, 3).copy()
    dt_bias = SQ_B + SQ_A * dtb                          # [2, DIN] per-channel bias
    dt_bias_t = dt_bias.reshape(ND, NDT, 128).transpose(2, 0, 1).copy()

    # B/C projection [DIN, 8]
    wbc = np.stack([xpw[i][:, DTR:] for i in range(ND)])  # [2, DIN, 8]
    wbc_t = wbc.reshape(ND, NDT, 128, 8).transpose(2, 0, 1, 3).copy()
    csc = np.array([1, 1, 1, 1, CSCALE, CSCALE, CSCALE, CSCALE],
                   np.float32).reshape(8, 1)

    # es: check the structured A pattern (A[d,s] = const_s per state)
    A = -np.exp(A_log)                                   # [2, DIN, 4]
    es_struct = all(
        np.allclose(A[i, :, s], A[i, 0, s], rtol=1e-6, atol=1e-7)
        for i in range(ND) for s in range(DST)
    )
    es_scale = A[:, 0, :].copy()                         # [2, 4] (structured)
    asc = np.transpose(A, (0, 2, 1))                     # [2, 4, DIN]
    asc_t = asc.reshape(ND, DST, NDT, 128).transpose(3, 0, 1, 2).copy()
    ascb_t = (asc_t * C_SP).copy()

    dp16 = (CSCALE * Dp).reshape(ND, NDT, 128).transpose(2, 0, 1).copy()

    # gw = out_proj @ dir_proj * dirw @ fusion_w1 slice, fp8
    gw = np.zeros((ND, 4, 128, 2 * D), np.float32)
    for i in range(ND):
        gwi = (opw[i] @ dpw[i] * dw[i]) @ fw1[i * D:(i + 1) * D, :]
        gw[i, :NDT] = gwi.reshape(NDT, 128, 2 * D)
    s_g = _pow2_scale(gw)
    gw *= s_g
    gw_t = np.transpose(gw, (2, 0, 1, 3)).copy()         # [128, dir, kt, 768]

    # w2 scaled by exactly RSCALE (residual also scaled by RSCALE)
    w2 = fw2 * RSCALE
    assert np.abs(w2).max() < 400.0, "w2*RSCALE overflows fp8"
    w2_t = w2.reshape(6, 128, D).transpose(1, 0, 2).copy()  # [128, 6kt, D]

    return {
        "host": {
            "s_xi": s_xi, "s_z": s_z, "s_g": s_g,
            "es_struct": es_struct, "es_scale": es_scale,
        },
        "sig": np.ascontiguousarray(sig.T).reshape(NDT, 128, L).transpose(1, 0, 2).copy().astype(BF),
        "wxik": wxik_t.astype(E4M3),
        "wz": wz_t.astype(E4M3),
        "wdtd": wdtd_t.astype(BF),
        "dtbias": dt_bias_t.astype(np.float32),
        "wbc": wbc_t.astype(BF),
        "csc": csc,
        "asc": asc_t.astype(np.float32),
        "ascb": ascb_t.astype(np.float32),
        "dp16": dp16.astype(np.float32),
        "gw": gw_t.astype(E4M3),
        "w2": w2_t.astype(E4M3),
        "lng": np.asarray(inp["ln_out_g"], np.float32)[None, :],
        "lnb": np.asarray(inp["ln_out_b"], np.float32)[None, :],
        "eye": np.eye(128, dtype=np.float32).astype(BF),
    }


def _flip32(ap2d, col0, ncols):
    """View of ap2d[:, col0:col0+ncols] with each 32-block reversed."""
    step = ap2d.ap[-1][0]
    return bass.AP(
        tensor=ap2d.tensor,
        offset=ap2d.offset + (col0 + 31) * step,
        ap=[list(ap2d.ap[0]), [32 * step, ncols // 32], [-step, 32]],
    )


def _flat(ap3d, n):
    """Flatten a contiguous [128, k, m] AP to [128, k*m]."""
    return bass.AP(
        tensor=ap3d.tensor, offset=ap3d.offset,
        ap=[list(ap3d.ap[0]), [1, n]],
    )


def _bcast_mid(ap2d, k):
    """[128, m] AP -> [128, k, m] with stride-0 middle dim."""
    return bass.AP(
        tensor=ap2d.tensor, offset=ap2d.offset,
        ap=[list(ap2d.ap[0]), [0, k], list(ap2d.ap[-1])],
    )


def build(nc, nb=BL, ln2_affine=False, es_struct=True, es_scale=None,
          s_xi=(1.0, 1.0), s_z=(1.0, 1.0), s_g=1.0):
    x_d = nc.dram_tensor("x", [nb, L, D], F32, kind="ExternalInput")
    sig_d = nc.dram_tensor("sig", [128, NDT, L], BF16, kind="ExternalInput")
    wxik_d = nc.dram_tensor("wxik", [128, ND, DCONV, 4, DIN], FP8,
                            kind="ExternalInput")
    wz_d = nc.dram_tensor("wz", [128, ND, 4, DIN], FP8, kind="ExternalInput")
    wdtd_d = nc.dram_tensor("wdtd", [128, ND, NDT, DIN], BF16,
                            kind="ExternalInput")
    dtbias_d = nc.dram_tensor("dtbias", [128, ND, NDT], F32, kind="ExternalInput")
    wbc_d = nc.dram_tensor("wbc", [128, ND, NDT, 8], BF16, kind="ExternalInput")
    csc_d = nc.dram_tensor("csc", [8, 1], F32, kind="ExternalInput")
    asc_d = nc.dram_tensor("asc", [128, ND, DST, NDT], F32, kind="ExternalInput")
    ascb_d = nc.dram_tensor("ascb", [128, ND, DST, NDT], F32, kind="ExternalInput")
    dp16_d = nc.dram_tensor("dp16", [128, ND, NDT], F32, kind="ExternalInput")
    gw_d = nc.dram_tensor("gw", [128, ND, 4, 2 * D], FP8, kind="ExternalInput")
    w2_d = nc.dram_tensor("w2", [128, 6, D], FP8, kind="ExternalInput")
    lng_d = nc.dram_tensor("lng", [1, D], F32, kind="ExternalInput")
    lnb_d = nc.dram_tensor("lnb", [1, D], F32, kind="ExternalInput")
    eye_d = nc.dram_tensor("eye", [128, 128], BF16, kind="ExternalInput")
    out_d = nc.dram_tensor("out", [nb, L, D], F32, kind="ExternalOutput")
    stg_d = nc.dram_tensor("bcstage", [nb, ND, 8, L], BF16, kind="Internal")

    with tile.TileContext(nc) as tc, ExitStack() as ctx:
        wp = ctx.enter_context(tc.tile_pool(name="wp", bufs=1))
        stat = ctx.enter_context(tc.tile_pool(name="stat", bufs=3))
        xls_p = ctx.enter_context(tc.tile_pool(name="xls", bufs=3))
        xin_p = ctx.enter_context(tc.tile_pool(name="xin", bufs=3))
        xc_p = ctx.enter_context(tc.tile_pool(name="xc", bufs=2))
        xcf_p = ctx.enter_context(tc.tile_pool(name="xcf", bufs=1))
        av2_p = ctx.enter_context(tc.tile_pool(name="av2", bufs=3))
        av1_p = ctx.enter_context(tc.tile_pool(name="av1", bufs=1))
        avb_p = ctx.enter_context(tc.tile_pool(name="avb", bufs=2))
        str_p = ctx.enter_context(tc.tile_pool(name="strm", bufs=2))
        es_p = ctx.enter_context(tc.tile_pool(name="esp", bufs=4))
        bc_p = ctx.enter_context(tc.tile_pool(name="bcp", bufs=2))
        bc8_p = ctx.enter_context(tc.tile_pool(name="bc8p", bufs=1))
        yn_p = ctx.enter_context(tc.tile_pool(name="ynp", bufs=1))
        sc_p = ctx.enter_context(tc.tile_pool(name="scp", bufs=1))
        ps_tr = ctx.enter_context(tc.tile_pool(name="pstr", bufs=1, space="PSUM"))
        ps_w2 = ctx.enter_context(tc.tile_pool(name="psw2", bufs=2, space="PSUM"))
        ps_b = ctx.enter_context(tc.tile_pool(name="psb", bufs=2, space="PSUM"))
        ps_c = ctx.enter_context(tc.tile_pool(name="psc", bufs=1, space="PSUM"))

        def dma(dst, src):
            nc.sync.dma_start(out=dst, in_=src)

        def wdma(dst, src):
            nc.scalar.dma_start(out=dst, in_=src)

        # ---- weights to SBUF ----
        wxik_s = wp.tile([128, ND, DCONV, 4, DIN], FP8, tag="wxik")
        wdma(wxik_s, wxik_d.ap())
        wz_s = wp.tile([128, ND, 4, DIN], FP8, tag="wz")
        wdma(wz_s, wz_d.ap())
        wdtd_s = wp.tile([128, ND, NDT, DIN], BF16, tag="wdtd")
        wdma(wdtd_s, wdtd_d.ap())
        dtbias_s = wp.tile([128, ND, NDT], F32, tag="dtbias")
        wdma(dtbias_s, dtbias_d.ap())
        wbc_s = wp.tile([128, ND, NDT, 8], BF16, tag="wbc")
        wdma(wbc_s, wbc_d.ap())
        csc_s = wp.tile([8, 1], F32, tag="csc")
        wdma(csc_s, csc_d.ap())
        asc_s = wp.tile([128, ND, DST, NDT], F32, tag="asc")
        wdma(asc_s, asc_d.ap())
        ascb_s = wp.tile([128, ND, DST, NDT], F32, tag="ascb")
        wdma(ascb_s, ascb_d.ap())
        dp16_s = wp.tile([128, ND, NDT], F32, tag="dp16")
        wdma(dp16_s, dp16_d.ap())
        gw_s = wp.tile([128, ND, 4, 2 * D], FP8, tag="gw")
        wdma(gw_s, gw_d.ap())
        w2_s = wp.tile([128, 6, D], FP8, tag="w2")
        wdma(w2_s, w2_d.ap())
        sig_s = wp.tile([128, NDT, L], BF16, tag="sig")
        wdma(sig_s, sig_d.ap())
        eye_s = wp.tile([128, 128], BF16, tag="eye")
        wdma(eye_s, eye_d.ap())
        eps_s = wp.tile([128, 1], F32, tag="eps")
        nc.vector.memset(eps_s, EPS)
        if ln2_affine:
            lng_s = wp.tile([128, D], F32, tag="lng")
            wdma(lng_s, bass.AP(tensor=lng_d, offset=0, ap=[[0, 128], [1, D]]))
            lnb_s = wp.tile([128, D], F32, tag="lnb")
            wdma(lnb_s, bass.AP(tensor=lnb_d, offset=0, ap=[[0, 128], [1, D]]))

        x_dram = x_d.ap().rearrange("b (tt p) d -> b p tt d", p=128)
        out_dram = out_d.ap().rearrange("b (tt p) d -> b p tt d", p=128)

        last_exp = [None]
        last_silu = [None]

        def gate(inst, fam=0):
            ch = last_exp if fam == 0 else last_silu
            if ch[0] is not None:
                add_dep_helper(inst.ins, ch[0].ins, sync=False,
                               reason="act-table-grouping")
            ch[0] = inst

        state = {}

        def emit_A_load(b):
            x_tm = xin_p.tile([128, NTT, D], F32, tag="x_tm")
            dma(x_tm, x_dram[b])
            state[b] = {"x_tm": x_tm}

        def emit_A(b):
            x_tm = state[b]["x_tm"]
            # ---- LN1 stats ----
            mv8 = stat.tile([128, NTT, 2], F32, tag="mv8")
            for tt in range(NTT):
                st6 = stat.tile([128, 6], F32, tag="st6")
                nc.vector.bn_stats(out=st6, in_=x_tm[:, tt, :])
                nc.vector.bn_aggr(out=mv8[:, tt, :], in_=st6)
            sd8 = stat.tile([128, NTT], F32, tag="sd8")
            i1 = nc.scalar.activation(sd8, mv8[:, :, 1], AF.Ln, bias=eps_s)
            gate(i1)
            rs8 = stat.tile([128, NTT], F32, tag="rs8")
            i2 = nc.scalar.activation(rs8, sd8, AF.Exp, scale=-0.5)
            gate(i2)
            nmr8 = stat.tile([128, NTT], F32, tag="nmr8")
            nc.vector.tensor_tensor(nmr8, mv8[:, :, 0], rs8, OP.mult)
            nc.vector.tensor_scalar_mul(nmr8, nmr8, -1.0)

            # ---- LN apply + sig + transpose ----
            xc_fm = xc_p.tile([128, 4, L + 2], FP8, tag="xc_fm")
            if b < 2:
                nc.vector.memset(xc_fm[:, 3, :], 0.0)
                nc.vector.memset(
                    bass.AP(tensor=xc_fm.tensor, offset=xc_fm[:, :, :].offset,
                            ap=[list(xc_fm[:, :, :].ap[0]), [L + 2, 4], [1, 2]]),
                    0.0,
                )
            for tt in range(NTT):
                xls = xls_p.tile([128, D], BF16, tag="xls")
                nc.scalar.activation(
                    xls, x_tm[:, tt, :], AF.Identity,
                    scale=rs8[:, tt:tt + 1], bias=nmr8[:, tt:tt + 1],
                )
                if b == 0 and tt % 2 == 1:
                    # w2 psum pool is idle until iter 1; use it to
                    # double-buffer the batch-0 ladder (prologue only)
                    pt = ps_w2.tile([128, D], F32, tag="w2o")
                else:
                    pt = ps_tr.tile([128, D], F32, tag="tr")
                for k in range(NDT):
                    # transpose as a plain matmul (out = xls_chunk.T @ eye),
                    # then accumulate the constant feature-major pos/beta term
                    nc.tensor.matmul(
                        pt[:, k * 128:(k + 1) * 128],
                        xls[:, k * 128:(k + 1) * 128], eye_s,
                        start=True, stop=False,
                    )
                    nc.tensor.matmul(
                        pt[:, k * 128:(k + 1) * 128],
                        eye_s, sig_s[:, k, tt * 128:(tt + 1) * 128],
                        start=False, stop=True,
                    )
                nc.scalar.activation(
                    bass.AP(tensor=xc_fm.tensor,
                            offset=xc_fm[:, :, :].offset + 2 + tt * 128,
                            ap=[list(xc_fm[:, :, :].ap[0]), [L + 2, 3], [1, 128]]),
                    pt, AF.Copy,
                )

            # ---- flipped copy for dir-1 ----
            xcf = xcf_p.tile([128, 4, L + 2], FP8, tag="xcf")
            if b < 2:
                nc.vector.memset(xcf[:, 3, :], 0.0)
                nc.vector.memset(
                    bass.AP(tensor=xcf.tensor, offset=xcf[:, :, :].offset,
                            ap=[list(xcf[:, :, :].ap[0]), [L + 2, 4], [1, 2]]),
                    0.0,
                )
            for k in range(NDT):
                nc.scalar.activation(
                    xcf[:, k, 2:2 + L], _flip32(xc_fm[:, k, :], 2, L), AF.Copy
                )

            state[b]["xc_fm"] = xc_fm
            state[b]["xcf"] = xcf

        def emit_B1(b, dirs=(0, 1)):
            st = state[b]
            xc_fm, xcf = st["xc_fm"], st["xcf"]
            # ---- in_proj xi (conv folded, fp8 DR) + silu ----
            xcv_t = st.setdefault("xcv_t", [None, None])
            for i in dirs:
                xsrc = xcf if i == 1 else xc_fm
                xcv = av2_p.tile([128, NDT, L], BF16, tag="xcv")
                xcv_t[i] = xcv
                for mt in range(NDT):
                    pt = ps_b.tile([128, 1024], F32, tag="big")
                    for ch in range(2):
                        first = True
                        for k in range(DCONV):
                            for p in range(2):
                                nc.tensor.matmul(
                                    pt[:, ch * 512:(ch + 1) * 512],
                                    wxik_s[:, i, k, 2 * p:2 * p + 2,
                                           mt * 128:(mt + 1) * 128],
                                    xsrc[:, 2 * p:2 * p + 2,
                                         k + ch * 512:k + ch * 512 + 512],
                                    start=first,
                                    stop=(k == DCONV - 1 and p == 1),
                                    perf_mode=PM.DoubleRow,
                                )
                                first = False
                    gate(nc.scalar.activation(
                        _flat(xcv[:, mt, :], L), _flat(pt[:, :], 1024),
                        AF.Silu, scale=1.0 / s_xi[i],
                    ), fam=1)
        def emit_B2(b):
            st = state[b]
            xc_fm, xcf = st["xc_fm"], st["xcf"]
            # ---- z (fp8 DR) + silu ----
            z_t = []
            for i in range(ND):
                xsrc = xcf if i == 1 else xc_fm
                z_s = avb_p.tile([128, NDT, L], BF16, tag="z")
                z_t.append(z_s)
                for mt in range(NDT):
                    pt = ps_b.tile([128, 1024], F32, tag="big")
                    for ch in range(2):
                        for p in range(2):
                            nc.tensor.matmul(
                                pt[:, ch * 512:(ch + 1) * 512],
                                wz_s[:, i, 2 * p:2 * p + 2,
                                     mt * 128:(mt + 1) * 128],
                                xsrc[:, 2 * p:2 * p + 2,
                                     2 + ch * 512:2 + ch * 512 + 512],
                                start=(p == 0), stop=(p == 1),
                                perf_mode=PM.DoubleRow,
                            )
                    gate(nc.scalar.activation(
                        _flat(z_s[:, mt, :], L), _flat(pt[:, :], 1024),
                        AF.Silu, scale=1.0 / s_z[i],
                    ), fam=1)

            st["z_t"] = z_t

        def emit_B3(b, dirs=(0, 1)):
            st = state[b]
            xcv_t = st["xcv_t"]
            # ---- B/C rows + broadcast via DRAM ----
            bc8_t = st.setdefault("bc8_t", [None, None])
            for i in dirs:
                xbc = bc_p.tile([8, L], BF16, tag="xbc")
                for ch in range(2):
                    pt = ps_c.tile([8, 512], F32, tag="bc")
                    for kt in range(NDT):
                        nc.tensor.matmul(
                            pt, wbc_s[:, i, kt, :],
                            xcv_t[i][:, kt, ch * 512:(ch + 1) * 512],
                            start=(kt == 0), stop=(kt == NDT - 1),
                        )
                    gate(nc.scalar.activation(
                        xbc[:, ch * 512:(ch + 1) * 512], pt, AF.Copy,
                        scale=csc_s,
                    ), fam=1)
                nc.sync.dma_start(out=stg_d.ap()[b, i], in_=xbc[:, :])
                bc8 = bc8_p.tile([128, 8, L], BF16, tag="bc8")
                bc8_t[i] = bc8
                src = bass.AP(
                    tensor=stg_d, offset=(b * ND + i) * 8 * L,
                    ap=[[0, 128], [L, 8], [1, L]],
                )
                nc.sync.dma_start(out=bc8, in_=src)

            # ---- dt (fused proj, Square softplus) ----
            dt_t = st.setdefault("dt_t", [None, None])
            for i in dirs:
                dt_b = avb_p.tile([128, NDT, L], BF16, tag="dt")
                dt_t[i] = dt_b
                for mt in range(NDT):
                    pt = ps_b.tile([128, 1024], F32, tag="big")
                    for ch in range(2):
                        for kt in range(NDT):
                            nc.tensor.matmul(
                                pt[:, ch * 512:(ch + 1) * 512],
                                wdtd_s[:, i, kt, mt * 128:(mt + 1) * 128],
                                xcv_t[i][:, kt, ch * 512:(ch + 1) * 512],
                                start=(kt == 0), stop=(kt == NDT - 1),
                            )
                    gate(nc.scalar.activation(
                        _flat(dt_b[:, mt, :], L), _flat(pt[:, :], 1024),
                        AF.Square, scale=SQ_A,
                        bias=dtbias_s[:, i, mt:mt + 1],
                    ), fam=1)



        def _emit_es(b, i, dt_b, first):
            tiles = []
            for s in range(DST):
                es = es_p.tile([128, NDT, L], FP8, tag="es")
                tiles.append(es)
                if first and s < 4:
                    # the pool's four ring buffers; the exp never writes col0
                    # of any slice, so these zeros persist for every later
                    # reuse (scan seam reset).
                    nc.vector.memset(
                        bass.AP(tensor=es.tensor, offset=es[:, :, :].offset,
                                ap=[list(es[:, :, :].ap[0]), [L, 3], [1, 1]]),
                        0.0,
                    )
                eap_o = bass.AP(
                    tensor=es.tensor, offset=es[:, :, :].offset + 1,
                    ap=[list(es[:, :, :].ap[0]), [L, 3], [1, L - 1]],
                )
                eap_i = bass.AP(
                    tensor=dt_b.tensor, offset=dt_b[:, :, :].offset + 1,
                    ap=[list(dt_b[:, :, :].ap[0]), [L, 3], [1, L - 1]],
                )
                if es_struct:
                    ie = nc.scalar.activation(
                        eap_o, eap_i, AF.Exp,
                        scale=float(es_scale[i][s]),
                        bias=ascb_s[:, 0, s, 0:1],
                    )
                    gate(ie)
                else:
                    for mt in range(NDT):
                        ie = nc.scalar.activation(
                            es[:, mt, 1:L], dt_b[:, mt, 1:L], AF.Exp,
                            scale=asc_s[:, i, s, mt:mt + 1],
                            bias=ascb_s[:, i, s, mt:mt + 1],
                        )
                        gate(ie)
            return tiles

        def emit_E(b):
            st = state[b]
            st["es0"] = _emit_es(b, 0, st["dt_t"][0], first=(b == 0))

        def emit_E2(b):
            st = state[b]
            st["es1"] = _emit_es(b, 1, st["dt_t"][1], first=False)

        def emit_C(b):
            st = state[b]
            xcv_t, z_t, bc8_t, dt_t = (
                st["xcv_t"], st["z_t"], st["bc8_t"], st["dt_t"]
            )
            # ---- per-dir scan chain ----
            y_nat = []
            for i in range(ND):
                xcv, z_s, dt_b, bc8 = xcv_t[i], z_t[i], dt_t[i], bc8_t[i]
                # xdt = (dt + C_SP) * xcv
                dtf = str_p.tile([128, NDT, L], BF16, tag="strm")
                nc.vector.tensor_scalar_add(
                    _flat(dtf[:, :, :], NDT * L), _flat(dt_b[:, :, :], NDT * L),
                    C_SP,
                )
                xdt = av1_p.tile([128, NDT, L], BF16, tag="xdt")
                nc.vector.tensor_tensor(
                    _flat(xdt[:, :, :], NDT * L), _flat(dtf[:, :, :], NDT * L),
                    _flat(xcv[:, :, :], NDT * L), OP.mult,
                )

                acc = av1_p.tile([128, NDT, L], BF16, tag="acc")
                es_tiles = st["es0"] if i == 0 else st["es1"]
                for s in range(DST):
                    es = es_tiles[s]
                    bx = str_p.tile([128, NDT, L], BF16, tag="strm")
                    nc.vector.tensor_tensor(
                        _flat(bx[:, :, :], NDT * L),
                        _flat(xdt[:, :, :], NDT * L),
                        _bcast_mid(bc8[:, s, :], NDT), OP.mult,
                    )
                    hs = str_p.tile([128, NDT, L], BF16, tag="strm")
                    nc.vector.tensor_tensor_scan(
                        _flat(hs[:, :, :], NDT * L),
                        _flat(es[:, :, :], NDT * L),
                        _flat(bx[:, :, :], NDT * L),
                        0.0, OP.mult, OP.add,
                    )
                    if s == 0:
                        nc.vector.tensor_tensor(
                            _flat(acc[:, :, :], NDT * L),
                            _flat(hs[:, :, :], NDT * L),
                            _bcast_mid(bc8[:, DST, :], NDT), OP.mult,
                        )
                    else:
                        hbc = str_p.tile([128, NDT, L], BF16, tag="strm")
                        nc.vector.tensor_tensor(
                            _flat(hbc[:, :, :], NDT * L),
                            _flat(hs[:, :, :], NDT * L),
                            _bcast_mid(bc8[:, DST + s, :], NDT), OP.mult,
                        )
                        nc.vector.tensor_tensor(
                            _flat(acc[:, :, :], NDT * L),
                            _flat(acc[:, :, :], NDT * L),
                            _flat(hbc[:, :, :], NDT * L), OP.add,
                        )

                # y = (acc + 16Dp*xcv) * z in bf16, then ACT cast to fp8
                # (dir-1 un-flip rides the cast's output AP)
                yn = yn_p.tile([128, 4, L], FP8, tag=f"yn{i}")
                y_nat.append(yn)
                if b < 2:
                    nc.vector.memset(yn[:, 3, :], 0.0)
                t1 = str_p.tile([128, NDT, L], BF16, tag="strm")
                for mt in range(NDT):
                    nc.vector.tensor_scalar_mul(
                        t1[:, mt, :], xcv[:, mt, :], dp16_s[:, i, mt:mt + 1]
                    )
                nc.vector.tensor_tensor(
                    _flat(t1[:, :, :], NDT * L), _flat(t1[:, :, :], NDT * L),
                    _flat(acc[:, :, :], NDT * L), OP.add,
                )
                if i == 0:
                    yout = bass.AP(
                        tensor=yn.tensor, offset=yn[:, :, :].offset,
                        ap=[list(yn[:, :, :].ap[0]), [L, 3], [1, L]],
                    )
                else:
                    yout = bass.AP(
                        tensor=yn.tensor, offset=yn[:, :, :].offset + 31,
                        ap=[list(yn[:, :, :].ap[0]), [L, 3], [32, 32], [-1, 32]],
                    )
                if b == nb - 1 and i == 1:
                    # last batch, dir 1: write fp8 directly from V (the ACT
                    # cast would head the serial drain chain while V idles)
                    nc.vector.tensor_tensor(
                        yout, _flat(t1[:, :, :], NDT * L),
                        _flat(z_s[:, :, :], NDT * L), OP.mult,
                    )
                else:
                    ybf = str_p.tile([128, NDT, L], BF16, tag="strm")
                    nc.vector.tensor_tensor(
                        _flat(ybf[:, :, :], NDT * L),
                        _flat(t1[:, :, :], NDT * L),
                        _flat(z_s[:, :, :], NDT * L), OP.mult,
                    )
                    nc.scalar.activation(
                        yout, _flat(ybf[:, :, :], NDT * L), AF.Copy,
                    )

            st["y_nat"] = y_nat

        def emit_D_proj(b):
            st = state[b]
            y_nat = st["y_nat"]
            # ---- gw (fp8 DR) -> silu -> scat fp8 ----
            scat = sc_p.tile([128, 6, L], FP8, tag="scat")
            st["scat"] = scat
            for jt in range(6):
                pt = ps_b.tile([128, 1024], F32, tag="big")
                for ch in range(2):
                    first = True
                    for i in range(ND):
                        for p in range(2):
                            nc.tensor.matmul(
                                pt[:, ch * 512:(ch + 1) * 512],
                                gw_s[:, i, 2 * p:2 * p + 2,
                                     jt * 128:(jt + 1) * 128],
                                y_nat[i][:, 2 * p:2 * p + 2,
                                         ch * 512:(ch + 1) * 512],
                                start=first, stop=(i == ND - 1 and p == 1),
                                perf_mode=PM.DoubleRow,
                            )
                            first = False
                gate(nc.scalar.activation(
                    _flat(scat[:, jt, :], L), _flat(pt[:, :], 1024),
                    AF.Silu, scale=1.0 / (CSCALE * s_g),
                ), fam=1)

        def emit_D_fin(b):
            st = state.pop(b)
            x_tm, scat = st["x_tm"], st["scat"]
            # ---- w2 (fp8 DR) + residual*64 + LN2 ----
            mv8b = stat.tile([128, NTT, 2], F32, tag="mv8b")
            for tt in range(NTT):
                pt = ps_w2.tile([128, D], F32, tag="w2o")
                for q in range(3):
                    nc.tensor.matmul(
                        pt,
                        scat[:, 2 * q:2 * q + 2, tt * 128:(tt + 1) * 128],
                        w2_s[:, 2 * q:2 * q + 2, :],
                        start=(q == 0), stop=(q == 2),
                        perf_mode=PM.DoubleRow,
                    )
                u = x_tm[:, tt, :]
                nc.vector.scalar_tensor_tensor(
                    u, u, RSCALE, pt, OP.mult, OP.add
                )
                st6 = stat.tile([128, 6], F32, tag="st6")
                nc.vector.bn_stats(out=st6, in_=u)
                nc.vector.bn_aggr(out=mv8b[:, tt, :], in_=st6)
            sd8b = stat.tile([128, NTT], F32, tag="sd8b")
            i6 = nc.scalar.activation(sd8b, mv8b[:, :, 1], AF.Ln, bias=eps_s)
            gate(i6)
            rs8b = stat.tile([128, NTT], F32, tag="rs8b")
            i7 = nc.scalar.activation(rs8b, sd8b, AF.Exp, scale=-0.5)
            gate(i7)
            nmr8b = stat.tile([128, NTT], F32, tag="nmr8b")
            nc.vector.tensor_tensor(nmr8b, mv8b[:, :, 0], rs8b, OP.mult)
            nc.vector.tensor_scalar_mul(nmr8b, nmr8b, -1.0)
            for tt in range(NTT):
                u = x_tm[:, tt, :]
                nc.scalar.activation(
                    u, u, AF.Identity, scale=rs8b[:, tt:tt + 1],
                    bias=nmr8b[:, tt:tt + 1],
                )
                if ln2_affine:
                    nc.vector.tensor_tensor(u, u, lng_s, OP.mult)
                    nc.vector.tensor_tensor(u, u, lnb_s, OP.add)
                if b == nb - 1:
                    dma(out_dram[b][:, tt, :], u)
            if b < nb - 1:
                dma(out_dram[b], x_tm)

        emit_A_load(0)
        emit_A(0)
        emit_B1(0, dirs=(0,))
        emit_B3(0, dirs=(0,))
        emit_B1(0, dirs=(1,))
        emit_B3(0, dirs=(1,))
        emit_B2(0)
        if nb > 1:
            emit_A_load(1)
        emit_E(0)
        if nb > 2:
            emit_A_load(2)
        for k in range(nb):
            if k + 1 < nb:
                emit_A(k + 1)
            emit_E2(k)
            if k >= 1:
                emit_D_fin(k - 1)
            if k + 1 < nb:
                emit_B1(k + 1, dirs=(0,))
                emit_B3(k + 1, dirs=(0,))
                emit_B1(k + 1, dirs=(1,))
                emit_B3(k + 1, dirs=(1,))
                emit_B2(k + 1)
            emit_C(k)
            if k + 1 < nb:
                emit_E(k + 1)
            emit_D_proj(k)
            if k + 3 < nb:
                emit_A_load(k + 3)
        emit_D_fin(nb - 1)

    return nc


def kernel(**inputs):
    x = np.asarray(inputs["x"], np.float32)
    w = _host_weights(inputs)
    h = w.pop("host")

    ln2_affine = not (
        np.allclose(w["lng"], 1.0) and np.allclose(w["lnb"], 0.0)
    )
    nc = bacc.Bacc("TRN2", target_bir_lowering=False, debug=False)
    build(nc, nb=BL, ln2_affine=ln2_affine, es_struct=h["es_struct"],
          es_scale=h["es_scale"], s_xi=h["s_xi"], s_z=h["s_z"], s_g=h["s_g"])
    nc.compile()

    in_maps = []
    for c in range(NCORES):
        m = {"x": np.ascontiguousarray(x[c * BL:(c + 1) * BL])}
        m.update(w)
        in_maps.append(m)

    res = bass_utils.run_bass_kernel_spmd(nc, in_maps, core_ids=list(range(NCORES)))
    out = np.concatenate([res.results[c]["out"] for c in range(NCORES)], axis=0)
    return out.astype(np.float32)


# revision 34
# speedup vs baseline: 1.0016x; 1.0016x over previous
"""Trainium2 Bass kernel for nn_MultiDirectionalSpatialScanner.

Bidirectional Mamba-style spatial scanner, B=32 H=W=32 D=384, d_state=4.
Sharding: data-parallel over batch across 8 cores (4 batches/core).

v2 design vs v1:
  - fp8e4 + DoubleRow matmuls for in_proj(conv-folded), z, gw, w2 with
    power-of-2 weight prescales folded into ACT eviction scales.
  - dt computed via one fused matmul (x_proj_dt @ dt_proj collapsed on host),
    softplus via the Square trick with per-channel bias.
  - B/C broadcast rows produced via a DRAM round-trip broadcast DMA instead
    of 32 PE matmuls + 32 ACT evictions.
  - states concatenated along free dim: one [128, 3*1024] scan per (dir, s)
    with es=0 at tile seams (resets the recurrence), wide elementwise ops.
  - LN applies and dtype casts on ACT (per-partition scale/bias); tables
    ordered so only ~4 ACT table switches occur per batch.
  - residual scaled by 64 (LN2 is scale-invariant) so w2's fp8 prescale
    needs no extra correction pass.
  - batched input/output/broadcast DMAs.
"""

import math
import numpy as np
from contextlib import ExitStack

import ml_dtypes
import concourse.bass as bass
import concourse.bacc as bacc
import concourse.tile as tile
from concourse.tile import add_dep_helper
from concourse import mybir
from concourse import bass_utils

F32 = mybir.dt.float32
BF16 = mybir.dt.bfloat16
FP8 = mybir.dt.float8e4
AF = mybir.ActivationFunctionType
OP = mybir.AluOpType
PM = mybir.MatmulPerfMode

B, Hh, Ww, D = 32, 32, 32, 384
L = Hh * Ww                 # 1024
ND, DST, DCONV, DIN, DTR = 2, 4, 3, 384, 24
NCORES = 8
BL = B // NCORES            # 4 batches per core
NDT = DIN // 128            # 3 feature tiles
NTT = L // 128              # 8 token tiles per batch
EPS = 1e-5
C_SP = 0.1931471806         # ln2 - 1/2 (softplus quad constant)
SQ_A = 0.3535533906         # 1/sqrt(8)
SQ_B = 0.7071067812         # 1/sqrt(2)
CSCALE = 16.0               # C-row prescale so yn fits fp8 nicely
RSCALE = 64.0               # residual/w2 prescale (LN2 scale-invariant)
BF = ml_dtypes.bfloat16
E4M3 = ml_dtypes.float8_e4m3


def _pow2_scale(w, target=240.0):
    """Power-of-two scale s so |w*s| <= target."""
    a = float(np.abs(w).max())
    if a == 0.0:
        return 1.0
    return 2.0 ** math.floor(math.log2(target / a))


def _pos_embed_np(H, W, Dm):
    ph = (np.arange(H, dtype=np.float32) / (H - 1)) * 2 - 1
    pw = (np.arange(W, dtype=np.float32) / (W - 1)) * 2 - 1
    gh, gw = np.meshgrid(ph, pw, indexing="ij")
    div = np.exp(np.arange(0, Dm, 2, dtype=np.float32) * (-math.log(10000.0) / Dm))
    d4 = div[::2]
    pe = np.zeros((H, W, Dm), np.float32)
    pe[:, :, 0::4] = np.sin(gh[..., None] * d4)
    pe[:, :, 1::4] = np.cos(gh[..., None] * d4)
    pe[:, :, 2::4] = np.sin(gw[..., None] * d4)
    pe[:, :, 3::4] = np.cos(gw[..., None] * d4)
    return pe.reshape(H * W, Dm)


def _host_weights(inp):
    g = np.asarray(inp["ln_in_g"], np.float32)
    bta = np.asarray(inp["ln_in_b"], np.float32)
    ipw = np.asarray(inp["in_proj_w"], np.float32)      # [2, D, 2*DIN]
    cw = np.asarray(inp["conv_w"], np.float32)          # [2, DIN, 3]
    xpw = np.asarray(inp["x_proj_w"], np.float32)       # [2, DIN, 32]
    dtw = np.asarray(inp["dt_proj_w"], np.float32)      # [2, 24, DIN]
    dtb = np.asarray(inp["dt_proj_b"], np.float32)      # [2, DIN]
    A_log = np.asarray(inp["A_log"], np.float32)        # [2, DIN, 4]
    Dp = np.asarray(inp["D_param"], np.float32)         # [2, DIN]
    opw = np.asarray(inp["out_proj_w"], np.float32)     # [2, DIN, D]
    dpw = np.asarray(inp["dir_proj_w"], np.float32)     # [2, D, D]
    fw1 = np.asarray(inp["fusion_w1"], np.float32)      # [2D, 2D]
    fw2 = np.asarray(inp["fusion_w2"], np.float32)      # [2D, D]
    dw = np.asarray(inp["dir_weights"], np.float32)     # [2]

    pe = _pos_embed_np(Hh, Ww, D)                       # [L, D]
    sig = (bta[None, :] + pe) / g[None, :]              # [L, D]

    # xi weights with LN gamma + conv tap folded, fp8 with per-dir scale.
    wxi = np.stack([g[:, None] * ipw[i][:, :DIN] for i in range(ND)])  # [2,D,DIN]
    wxik = np.zeros((ND, DCONV, 4, 128, DIN), np.float32)  # [dir,tap,kt(pad),128,m]
    for i in range(ND):
        for k in range(DCONV):
            wk = wxi[i] * cw[i][None, :, k]             # [D(K), DIN(M)]
            wxik[i, k, :NDT] = wk.reshape(NDT, 128, DIN)
    s_xi = [_pow2_scale(wxik[i]) for i in range(ND)]
    for i in range(ND):
        wxik[i] *= s_xi[i]
    # layout [128, dir, tap, kt, M]
    wxik_t = np.transpose(wxik, (3, 0, 1, 2, 4)).copy()

    wz = np.zeros((ND, 4, 128, DIN), np.float32)
    for i in range(ND):
        wz[i, :NDT] = (g[:, None] * ipw[i][:, DIN:]).reshape(NDT, 128, DIN)
    s_z = [_pow2_scale(wz[i]) for i in range(ND)]
    for i in range(ND):
        wz[i] *= s_z[i]
    wz_t = np.transpose(wz, (2, 0, 1, 3)).copy()        # [128, dir, kt, M]

    # fused dt projection: xcv @ (xpw_dt @ dtw) + dtb, softplus-quad bias
    wdtd = np.stack([xpw[i][:, :DTR] @ dtw[i] for i in range(ND)])  # [2, DIN, DIN]
    wdtd_t = wdtd.reshape(ND, NDT, 128, DIN).transpose(2, 0, 1, 3).copy()
    dt_bias = SQ_B + SQ_A * dtb                          # [2, DIN] per-channel bias
    dt_bias_t = dt_bias.reshape(ND, NDT, 128).transpose(2, 0, 1).copy()

    # B/C projection [DIN, 8]
    wbc = np.stack([xpw[i][:, DTR:] for i in range(ND)])  # [2, DIN, 8]
    wbc_t = wbc.reshape(ND, NDT, 128, 8).transpose(2, 0, 1, 3).copy()
    csc = np.array([1, 1, 1, 1, CSCALE, CSCALE, CSCALE, CSCALE],
                   np.float32).reshape(8, 1)

    # es: check the structured A pattern (A[d,s] = const_s per state)
    A = -np.exp(A_log)                                   # [2, DIN, 4]
    es_struct = all(
        np.allclose(A[i, :, s], A[i, 0, s], rtol=1e-6, atol=1e-7)
        for i in range(ND) for s in range(DST)
    )
    es_scale = A[:, 0, :].copy()                         # [2, 4] (structured)
    asc = np.transpose(A, (0, 2, 1))                     # [2, 4, DIN]
    asc_t = asc.reshape(ND, DST, NDT, 128).transpose(3, 0, 1, 2).copy()
    ascb_t = (asc_t * C_SP).copy()

    dp16 = (CSCALE * Dp).reshape(ND, NDT, 128).transpose(2, 0, 1).copy()

    # gw = out_proj @ dir_proj * dirw @ fusion_w1 slice, fp8
    gw = np.zeros((ND, 4, 128, 2 * D), np.float32)
    for i in range(ND):
        gwi = (opw[i] @ dpw[i] * dw[i]) @ fw1[i * D:(i + 1) * D, :]
        gw[i, :NDT] = gwi.reshape(NDT, 128, 2 * D)
    s_g = _pow2_scale(gw)
    gw *= s_g
    gw_t = np.transpose(gw, (2, 0, 1, 3)).copy()         # [128, dir, kt, 768]

    # w2 scaled by exactly RSCALE (residual also scaled by RSCALE)
    w2 = fw2 * RSCALE
    assert np.abs(w2).max() < 400.0, "w2*RSCALE overflows fp8"
    w2_t = w2.reshape(6, 128, D).transpose(1, 0, 2).copy()  # [128, 6kt, D]

    return {
        "host": {
            "s_xi": s_xi, "s_z": s_z, "s_g": s_g,
            "es_struct": es_struct, "es_scale": es_scale,
        },
        "sig": np.ascontiguousarray(sig.T).reshape(NDT, 128, L).transpose(1, 0, 2).copy().astype(BF),
        "wxik": wxik_t.astype(E4M3),
        "wz": wz_t.astype(E4M3),
        "wdtd": wdtd_t.astype(BF),
        "dtbias": dt_bias_t.astype(np.float32),
        "wbc": wbc_t.astype(BF),
        "csc": csc,
        "asc": asc_t.astype(np.float32),
        "ascb": ascb_t.astype(np.float32),
        "dp16": dp16.astype(np.float32),
        "gw": gw_t.astype(E4M3),
        "w2": w2_t.astype(E4M3),
        "lng": np.asarray(inp["ln_out_g"], np.float32)[None, :],
        "lnb": np.asarray(inp["ln_out_b"], np.float32)[None, :],
        "eye": np.eye(128, dtype=np.float32).astype(BF),
    }


def _flip32(ap2d, col0, ncols):
    """View of ap2d[:, col0:col0+ncols] with each 32-block reversed."""
    step = ap2d.ap[-1][0]
    return bass.AP(
        tensor=ap2d.tensor,
        offset=ap2d.offset + (col0 + 31) * step,
        ap=[list(ap2d.ap[0]), [32 * step, ncols // 32], [-step, 32]],
    )


def _flat(ap3d, n):
    """Flatten a contiguous [128, k, m] AP to [128, k*m]."""
    return bass.AP(
        tensor=ap3d.tensor, offset=ap3d.offset,
        ap=[list(ap3d.ap[0]), [1, n]],
    )


def _bcast_mid(ap2d, k):
    """[128, m] AP -> [128, k, m] with stride-0 middle dim."""
    return bass.AP(
        tensor=ap2d.tensor, offset=ap2d.offset,
        ap=[list(ap2d.ap[0]), [0, k], list(ap2d.ap[-1])],
    )


def build(nc, nb=BL, ln2_affine=False, es_struct=True, es_scale=None,
          s_xi=(1.0, 1.0), s_z=(1.0, 1.0), s_g=1.0):
    x_d = nc.dram_tensor("x", [nb, L, D], F32, kind="ExternalInput")
    sig_d = nc.dram_tensor("sig", [128, NDT, L], BF16, kind="ExternalInput")
    wxik_d = nc.dram_tensor("wxik", [128, ND, DCONV, 4, DIN], FP8,
                            kind="ExternalInput")
    wz_d = nc.dram_tensor("wz", [128, ND, 4, DIN], FP8, kind="ExternalInput")
    wdtd_d = nc.dram_tensor("wdtd", [128, ND, NDT, DIN], BF16,
                            kind="ExternalInput")
    dtbias_d = nc.dram_tensor("dtbias", [128, ND, NDT], F32, kind="ExternalInput")
    wbc_d = nc.dram_tensor("wbc", [128, ND, NDT, 8], BF16, kind="ExternalInput")
    csc_d = nc.dram_tensor("csc", [8, 1], F32, kind="ExternalInput")
    asc_d = nc.dram_tensor("asc", [128, ND, DST, NDT], F32, kind="ExternalInput")
    ascb_d = nc.dram_tensor("ascb", [128, ND, DST, NDT], F32, kind="ExternalInput")
    dp16_d = nc.dram_tensor("dp16", [128, ND, NDT], F32, kind="ExternalInput")
    gw_d = nc.dram_tensor("gw", [128, ND, 4, 2 * D], FP8, kind="ExternalInput")
    w2_d = nc.dram_tensor("w2", [128, 6, D], FP8, kind="ExternalInput")
    lng_d = nc.dram_tensor("lng", [1, D], F32, kind="ExternalInput")
    lnb_d = nc.dram_tensor("lnb", [1, D], F32, kind="ExternalInput")
    eye_d = nc.dram_tensor("eye", [128, 128], BF16, kind="ExternalInput")
    out_d = nc.dram_tensor("out", [nb, L, D], F32, kind="ExternalOutput")
    stg_d = nc.dram_tensor("bcstage", [nb, ND, 8, L], BF16, kind="Internal")

    with tile.TileContext(nc) as tc, ExitStack() as ctx:
        wp = ctx.enter_context(tc.tile_pool(name="wp", bufs=1))
        stat = ctx.enter_context(tc.tile_pool(name="stat", bufs=3))
        xls_p = ctx.enter_context(tc.tile_pool(name="xls", bufs=3))
        xin_p = ctx.enter_context(tc.tile_pool(name="xin", bufs=3))
        xc_p = ctx.enter_context(tc.tile_pool(name="xc", bufs=2))
        xcf_p = ctx.enter_context(tc.tile_pool(name="xcf", bufs=1))
        av2_p = ctx.enter_context(tc.tile_pool(name="av2", bufs=3))
        av1_p = ctx.enter_context(tc.tile_pool(name="av1", bufs=1))
        avb_p = ctx.enter_context(tc.tile_pool(name="avb", bufs=2))
        str_p = ctx.enter_context(tc.tile_pool(name="strm", bufs=2))
        es_p = ctx.enter_context(tc.tile_pool(name="esp", bufs=4))
        bc_p = ctx.enter_context(tc.tile_pool(name="bcp", bufs=2))
        bc8_p = ctx.enter_context(tc.tile_pool(name="bc8p", bufs=1))
        yn_p = ctx.enter_context(tc.tile_pool(name="ynp", bufs=1))
        sc_p = ctx.enter_context(tc.tile_pool(name="scp", bufs=1))
        ps_tr = ctx.enter_context(tc.tile_pool(name="pstr", bufs=1, space="PSUM"))
        ps_w2 = ctx.enter_context(tc.tile_pool(name="psw2", bufs=2, space="PSUM"))
        ps_b = ctx.enter_context(tc.tile_pool(name="psb", bufs=2, space="PSUM"))
        ps_c = ctx.enter_context(tc.tile_pool(name="psc", bufs=1, space="PSUM"))

        def dma(dst, src):
            nc.sync.dma_start(out=dst, in_=src)

        def wdma(dst, src):
            nc.scalar.dma_start(out=dst, in_=src)

        # ---- weights to SBUF ----
        wxik_s = wp.tile([128, ND, DCONV, 4, DIN], FP8, tag="wxik")
        wdma(wxik_s, wxik_d.ap())
        wz_s = wp.tile([128, ND, 4, DIN], FP8, tag="wz")
        wdma(wz_s, wz_d.ap())
        wdtd_s = wp.tile([128, ND, NDT, DIN], BF16, tag="wdtd")
        wdma(wdtd_s, wdtd_d.ap())
        dtbias_s = wp.tile([128, ND, NDT], F32, tag="dtbias")
        wdma(dtbias_s, dtbias_d.ap())
        wbc_s = wp.tile([128, ND, NDT, 8], BF16, tag="wbc")
        wdma(wbc_s, wbc_d.ap())
        csc_s = wp.tile([8, 1], F32, tag="csc")
        wdma(csc_s, csc_d.ap())
        asc_s = wp.tile([128, ND, DST, NDT], F32, tag="asc")
        wdma(asc_s, asc_d.ap())
        ascb_s = wp.tile([128, ND, DST, NDT], F32, tag="ascb")
        wdma(ascb_s, ascb_d.ap())
        dp16_s = wp.tile([128, ND, NDT], F32, tag="dp16")
        wdma(dp16_s, dp16_d.ap())
        gw_s = wp.tile([128, ND, 4, 2 * D], FP8, tag="gw")
        wdma(gw_s, gw_d.ap())
        w2_s = wp.tile([128, 6, D], FP8, tag="w2")
        wdma(w2_s, w2_d.ap())
        sig_s = wp.tile([128, NDT, L], BF16, tag="sig")
        wdma(sig_s, sig_d.ap())
        eye_s = wp.tile([128, 128], BF16, tag="eye")
        wdma(eye_s, eye_d.ap())
        eps_s = wp.tile([128, 1], F32, tag="eps")
        nc.vector.memset(eps_s, EPS)
        if ln2_affine:
            lng_s = wp.tile([128, D], F32, tag="lng")
            wdma(lng_s, bass.AP(tensor=lng_d, offset=0, ap=[[0, 128], [1, D]]))
            lnb_s = wp.tile([128, D], F32, tag="lnb")
            wdma(lnb_s, bass.AP(tensor=lnb_d, offset=0, ap=[[0, 128], [1, D]]))

        x_dram = x_d.ap().rearrange("b (tt p) d -> b p tt d", p=128)
        out_dram = out_d.ap().rearrange("b (tt p) d -> b p tt d", p=128)

        last_exp = [None]
        last_silu = [None]

        def gate(inst, fam=0):
            ch = last_exp if fam == 0 else last_silu
            if ch[0] is not None:
                add_dep_helper(inst.ins, ch[0].ins, sync=False,
                               reason="act-table-grouping")
            ch[0] = inst

        state = {}

        def emit_A_load(b):
            x_tm = xin_p.tile([128, NTT, D], F32, tag="x_tm")
            dma(x_tm, x_dram[b])
            state[b] = {"x_tm": x_tm}

        def emit_A(b):
            x_tm = state[b]["x_tm"]
            # ---- LN1 stats ----
            mv8 = stat.tile([128, NTT, 2], F32, tag="mv8")
            for tt in range(NTT):
                st6 = stat.tile([128, 6], F32, tag="st6")
                nc.vector.bn_stats(out=st6, in_=x_tm[:, tt, :])
                nc.vector.bn_aggr(out=mv8[:, tt, :], in_=st6)
            sd8 = stat.tile([128, NTT], F32, tag="sd8")
            i1 = nc.scalar.activation(sd8, mv8[:, :, 1], AF.Ln, bias=eps_s)
            gate(i1)
            rs8 = stat.tile([128, NTT], F32, tag="rs8")
            i2 = nc.scalar.activation(rs8, sd8, AF.Exp, scale=-0.5)
            gate(i2)
            nmr8 = stat.tile([128, NTT], F32, tag="nmr8")
            nc.vector.tensor_tensor(nmr8, mv8[:, :, 0], rs8, OP.mult)
            nc.vector.tensor_scalar_mul(nmr8, nmr8, -1.0)

            # ---- LN apply + sig + transpose ----
            xc_fm = xc_p.tile([128, 4, L + 2], FP8, tag="xc_fm")
            if b < 2:
                nc.vector.memset(xc_fm[:, 3, :], 0.0)
                nc.vector.memset(
                    bass.AP(tensor=xc_fm.tensor, offset=xc_fm[:, :, :].offset,
                            ap=[list(xc_fm[:, :, :].ap[0]), [L + 2, 4], [1, 2]]),
                    0.0,
                )
            for tt in range(NTT):
                xls = xls_p.tile([128, D], BF16, tag="xls")
                nc.scalar.activation(
                    xls, x_tm[:, tt, :], AF.Identity,
                    scale=rs8[:, tt:tt + 1], bias=nmr8[:, tt:tt + 1],
                )
                if b == 0 and tt % 2 == 1:
                    # w2 psum pool is idle until iter 1; use it to
                    # double-buffer the batch-0 ladder (prologue only)
                    pt = ps_w2.tile([128, D], F32, tag="w2o")
                else:
                    pt = ps_tr.tile([128, D], F32, tag="tr")
                for k in range(NDT):
                    # transpose as a plain matmul (out = xls_chunk.T @ eye),
                    # then accumulate the constant feature-major pos/beta term
                    nc.tensor.matmul(
                        pt[:, k * 128:(k + 1) * 128],
                        xls[:, k * 128:(k + 1) * 128], eye_s,
                        start=True, stop=False,
                    )
                    nc.tensor.matmul(
                        pt[:, k * 128:(k + 1) * 128],
                        eye_s, sig_s[:, k, tt * 128:(tt + 1) * 128],
                        start=False, stop=True,
                    )
                nc.scalar.activation(
                    bass.AP(tensor=xc_fm.tensor,
                            offset=xc_fm[:, :, :].offset + 2 + tt * 128,
                            ap=[list(xc_fm[:, :, :].ap[0]), [L + 2, 3], [1, 128]]),
                    pt, AF.Copy,
                )

            # ---- flipped copy for dir-1 ----
            xcf = xcf_p.tile([128, 4, L + 2], FP8, tag="xcf")
            if b < 2:
                nc.vector.memset(xcf[:, 3, :], 0.0)
                nc.vector.memset(
                    bass.AP(tensor=xcf.tensor, offset=xcf[:, :, :].offset,
                            ap=[list(xcf[:, :, :].ap[0]), [L + 2, 4], [1, 2]]),
                    0.0,
                )
            for k in range(NDT):
                nc.scalar.activation(
                    xcf[:, k, 2:2 + L], _flip32(xc_fm[:, k, :], 2, L), AF.Copy
                )

            state[b]["xc_fm"] = xc_fm
            state[b]["xcf"] = xcf

        def emit_B1(b, dirs=(0, 1)):
            st = state[b]
            xc_fm, xcf = st["xc_fm"], st["xcf"]
            # ---- in_proj xi (conv folded, fp8 DR) + silu ----
            xcv_t = st.setdefault("xcv_t", [None, None])
            for i in dirs:
                xsrc = xcf if i == 1 else xc_fm
                xcv = av2_p.tile([128, NDT, L], BF16, tag="xcv")
                xcv_t[i] = xcv
                for mt in range(NDT):
                    pt = ps_b.tile([128, 1024], F32, tag="big")
                    for ch in range(2):
                        first = True
                        for k in range(DCONV):
                            for p in range(2):
                                nc.tensor.matmul(
                                    pt[:, ch * 512:(ch + 1) * 512],
                                    wxik_s[:, i, k, 2 * p:2 * p + 2,
                                           mt * 128:(mt + 1) * 128],
                                    xsrc[:, 2 * p:2 * p + 2,
                                         k + ch * 512:k + ch * 512 + 512],
                                    start=first,
                                    stop=(k == DCONV - 1 and p == 1),
                                    perf_mode=PM.DoubleRow,
                                )
                                first = False
                    gate(nc.scalar.activation(
                        _flat(xcv[:, mt, :], L), _flat(pt[:, :], 1024),
                        AF.Silu, scale=1.0 / s_xi[i],
                    ), fam=1)
        def emit_B2(b):
            st = state[b]
            xc_fm, xcf = st["xc_fm"], st["xcf"]
            # ---- z (fp8 DR) + silu ----
            z_t = []
            for i in range(ND):
                xsrc = xcf if i == 1 else xc_fm
                z_s = avb_p.tile([128, NDT, L], BF16, tag="z")
                z_t.append(z_s)
                for mt in range(NDT):
                    pt = ps_b.tile([128, 1024], F32, tag="big")
                    for ch in range(2):
                        for p in range(2):
                            nc.tensor.matmul(
                                pt[:, ch * 512:(ch + 1) * 512],
                                wz_s[:, i, 2 * p:2 * p + 2,
                                     mt * 128:(mt + 1) * 128],
                                xsrc[:, 2 * p:2 * p + 2,
                                     2 + ch * 512:2 + ch * 512 + 512],
                                start=(p == 0), stop=(p == 1),
                                perf_mode=PM.DoubleRow,
                            )
                    gate(nc.scalar.activation(
                        _flat(z_s[:, mt, :], L), _flat(pt[:, :], 1024),
                        AF.Silu, scale=1.0 / s_z[i],
                    ), fam=1)

            st["z_t"] = z_t

        def emit_B3(b, dirs=(0, 1)):
            st = state[b]
            xcv_t = st["xcv_t"]
            # ---- B/C rows + broadcast via DRAM ----
            bc8_t = st.setdefault("bc8_t", [None, None])
            for i in dirs:
                xbc = bc_p.tile([8, L], BF16, tag="xbc")
                for ch in range(2):
                    pt = ps_c.tile([8, 512], F32, tag="bc")
                    for kt in range(NDT):
                        nc.tensor.matmul(
                            pt, wbc_s[:, i, kt, :],
                            xcv_t[i][:, kt, ch * 512:(ch + 1) * 512],
                            start=(kt == 0), stop=(kt == NDT - 1),
                        )
                    gate(nc.scalar.activation(
                        xbc[:, ch * 512:(ch + 1) * 512], pt, AF.Copy,
                        scale=csc_s,
                    ), fam=1)
                nc.sync.dma_start(out=stg_d.ap()[b, i], in_=xbc[:, :])
                bc8 = bc8_p.tile([128, 8, L], BF16, tag="bc8")
                bc8_t[i] = bc8
                src = bass.AP(
                    tensor=stg_d, offset=(b * ND + i) * 8 * L,
                    ap=[[0, 128], [L, 8], [1, L]],
                )
                nc.sync.dma_start(out=bc8, in_=src)

            # ---- dt (fused proj, Square softplus) ----
            dt_t = st.setdefault("dt_t", [None, None])
            for i in dirs:
                dt_b = avb_p.tile([128, NDT, L], BF16, tag="dt")
                dt_t[i] = dt_b
                for mt in range(NDT):
                    pt = ps_b.tile([128, 1024], F32, tag="big")
                    for ch in range(2):
                        for kt in range(NDT):
                            nc.tensor.matmul(
                                pt[:, ch * 512:(ch + 1) * 512],
                                wdtd_s[:, i, kt, mt * 128:(mt + 1) * 128],
                                xcv_t[i][:, kt, ch * 512:(ch + 1) * 512],
                                start=(kt == 0), stop=(kt == NDT - 1),
                            )
                    gate(nc.scalar.activation(
                        _flat(dt_b[:, mt, :], L), _flat(pt[:, :], 1024),
                        AF.Square, scale=SQ_A,
                        bias=dtbias_s[:, i, mt:mt + 1],
                    ), fam=1)



        def _emit_es(b, i, dt_b, first):
            tiles = []
            for s in range(DST):
                es = es_p.tile([128, NDT, L], FP8, tag="es")
                tiles.append(es)
                if first and s < 4:
                    # the pool's four ring buffers; the exp never writes col0
                    # of any slice, so these zeros persist for every later
                    # reuse (scan seam reset).
                    nc.vector.memset(
                        bass.AP(tensor=es.tensor, offset=es[:, :, :].offset,
                                ap=[list(es[:, :, :].ap[0]), [L, 3], [1, 1]]),
                        0.0,
                    )
                eap_o = bass.AP(
                    tensor=es.tensor, offset=es[:, :, :].offset + 1,
                    ap=[list(es[:, :, :].ap[0]), [L, 3], [1, L - 1]],
                )
                eap_i = bass.AP(
                    tensor=dt_b.tensor, offset=dt_b[:, :, :].offset + 1,
                    ap=[list(dt_b[:, :, :].ap[0]), [L, 3], [1, L - 1]],
                )
                if es_struct:
                    ie = nc.scalar.activation(
                        eap_o, eap_i, AF.Exp,
                        scale=float(es_scale[i][s]),
                        bias=ascb_s[:, 0, s, 0:1],
                    )
                    gate(ie)
                else:
                    for mt in range(NDT):
                        ie = nc.scalar.activation(
                            es[:, mt, 1:L], dt_b[:, mt, 1:L], AF.Exp,
                            scale=asc_s[:, i, s, mt:mt + 1],
                            bias=ascb_s[:, i, s, mt:mt + 1],
                        )
                        gate(ie)
            return tiles

        def emit_E(b):
            st = state[b]
            st["es0"] = _emit_es(b, 0, st["dt_t"][0], first=(b == 0))

        def emit_E2(b):
            st = state[b]
            st["es1"] = _emit_es(b, 1, st["dt_t"][1], first=False)

        def emit_C(b):
            st = state[b]
            xcv_t, z_t, bc8_t, dt_t = (
                st["xcv_t"], st["z_t"], st["bc8_t"], st["dt_t"]
            )
            # ---- per-dir scan chain ----
            y_nat = []
            for i in range(ND):
                xcv, z_s, dt_b, bc8 = xcv_t[i], z_t[i], dt_t[i], bc8_t[i]
                # xdt = (dt + C_SP) * xcv
                dtf = str_p.tile([128, NDT, L], BF16, tag="strm")
                nc.vector.tensor_scalar_add(
                    _flat(dtf[:, :, :], NDT * L), _flat(dt_b[:, :, :], NDT * L),
                    C_SP,
                )
                xdt = av1_p.tile([128, NDT, L], BF16, tag="xdt")
                nc.vector.tensor_tensor(
                    _flat(xdt[:, :, :], NDT * L), _flat(dtf[:, :, :], NDT * L),
                    _flat(xcv[:, :, :], NDT * L), OP.mult,
                )

                acc = av1_p.tile([128, NDT, L], BF16, tag="acc")
                es_tiles = st["es0"] if i == 0 else st["es1"]
                for s in range(DST):
                    es = es_tiles[s]
                    bx = str_p.tile([128, NDT, L], BF16, tag="strm")
                    nc.vector.tensor_tensor(
                        _flat(bx[:, :, :], NDT * L),
                        _flat(xdt[:, :, :], NDT * L),
                        _bcast_mid(bc8[:, s, :], NDT), OP.mult,
                    )
                    hs = str_p.tile([128, NDT, L], BF16, tag="strm")
                    nc.vector.tensor_tensor_scan(
                        _flat(hs[:, :, :], NDT * L),
                        _flat(es[:, :, :], NDT * L),
                        _flat(bx[:, :, :], NDT * L),
                        0.0, OP.mult, OP.add,
                    )
                    if s == 0:
                        nc.vector.tensor_tensor(
                            _flat(acc[:, :, :], NDT * L),
                            _flat(hs[:, :, :], NDT * L),
                            _bcast_mid(bc8[:, DST, :], NDT), OP.mult,
                        )
                    else:
                        hbc = str_p.tile([128, NDT, L], BF16, tag="strm")
                        nc.vector.tensor_tensor(
                            _flat(hbc[:, :, :], NDT * L),
                            _flat(hs[:, :, :], NDT * L),
                            _bcast_mid(bc8[:, DST + s, :], NDT), OP.mult,
                        )
                        nc.vector.tensor_tensor(
                            _flat(acc[:, :, :], NDT * L),
                            _flat(acc[:, :, :], NDT * L),
                            _flat(hbc[:, :, :], NDT * L), OP.add,
                        )

                # y = (acc + 16Dp*xcv) * z in bf16, then ACT cast to fp8
                # (dir-1 un-flip rides the cast's output AP)
                yn = yn_p.tile([128, 4, L], FP8, tag=f"yn{i}")
                y_nat.append(yn)
                if b < 2:
                    nc.vector.memset(yn[:, 3, :], 0.0)
                t1 = str_p.tile([128, NDT, L], BF16, tag="strm")
                for mt in range(NDT):
                    nc.vector.tensor_scalar_mul(
                        t1[:, mt, :], xcv[:, mt, :], dp16_s[:, i, mt:mt + 1]
                    )
                nc.vector.tensor_tensor(
                    _flat(t1[:, :, :], NDT * L), _flat(t1[:, :, :], NDT * L),
                    _flat(acc[:, :, :], NDT * L), OP.add,
                )
                ybf = str_p.tile([128, NDT, L], BF16, tag="strm")
                nc.vector.tensor_tensor(
                    _flat(ybf[:, :, :], NDT * L), _flat(t1[:, :, :], NDT * L),
                    _flat(z_s[:, :, :], NDT * L), OP.mult,
                )
                if i == 0:
                    yout = bass.AP(
                        tensor=yn.tensor, offset=yn[:, :, :].offset,
                        ap=[list(yn[:, :, :].ap[0]), [L, 3], [1, L]],
                    )
                else:
                    yout = bass.AP(
                        tensor=yn.tensor, offset=yn[:, :, :].offset + 31,
                        ap=[list(yn[:, :, :].ap[0]), [L, 3], [32, 32], [-1, 32]],
                    )
                nc.scalar.activation(
                    yout, _flat(ybf[:, :, :], NDT * L), AF.Copy,
                )

            st["y_nat"] = y_nat

        def emit_D_proj(b):
            st = state[b]
            y_nat = st["y_nat"]
            # ---- gw (fp8 DR) -> silu -> scat fp8 ----
            scat = sc_p.tile([128, 6, L], FP8, tag="scat")
            st["scat"] = scat
            for jt in range(6):
                pt = ps_b.tile([128, 1024], F32, tag="big")
                for ch in range(2):
                    first = True
                    for i in range(ND):
                        for p in range(2):
                            nc.tensor.matmul(
                                pt[:, ch * 512:(ch + 1) * 512],
                                gw_s[:, i, 2 * p:2 * p + 2,
                                     jt * 128:(jt + 1) * 128],
                                y_nat[i][:, 2 * p:2 * p + 2,
                                         ch * 512:(ch + 1) * 512],
                                start=first, stop=(i == ND - 1 and p == 1),
                                perf_mode=PM.DoubleRow,
                            )
                            first = False
                gate(nc.scalar.activation(
                    _flat(scat[:, jt, :], L), _flat(pt[:, :], 1024),
                    AF.Silu, scale=1.0 / (CSCALE * s_g),
                ), fam=1)

        def emit_D_fin(b):
            st = state.pop(b)
            x_tm, scat = st["x_tm"], st["scat"]
            # ---- w2 (fp8 DR) + residual*64 + LN2 ----
            mv8b = stat.tile([128, NTT, 2], F32, tag="mv8b")
            for tt in range(NTT):
                pt = ps_w2.tile([128, D], F32, tag="w2o")
                for q in range(3):
                    nc.tensor.matmul(
                        pt,
                        scat[:, 2 * q:2 * q + 2, tt * 128:(tt + 1) * 128],
                        w2_s[:, 2 * q:2 * q + 2, :],
                        start=(q == 0), stop=(q == 2),
                        perf_mode=PM.DoubleRow,
                    )
                u = x_tm[:, tt, :]
                nc.vector.scalar_tensor_tensor(
                    u, u, RSCALE, pt, OP.mult, OP.add
                )
                st6 = stat.tile([128, 6], F32, tag="st6")
                nc.vector.bn_stats(out=st6, in_=u)
                nc.vector.bn_aggr(out=mv8b[:, tt, :], in_=st6)
            sd8b = stat.tile([128, NTT], F32, tag="sd8b")
            i6 = nc.scalar.activation(sd8b, mv8b[:, :, 1], AF.Ln, bias=eps_s)
            gate(i6)
            rs8b = stat.tile([128, NTT], F32, tag="rs8b")
            i7 = nc.scalar.activation(rs8b, sd8b, AF.Exp, scale=-0.5)
            gate(i7)
            nmr8b = stat.tile([128, NTT], F32, tag="nmr8b")
            nc.vector.tensor_tensor(nmr8b, mv8b[:, :, 0], rs8b, OP.mult)
            nc.vector.tensor_scalar_mul(nmr8b, nmr8b, -1.0)
            for tt in range(NTT):
                u = x_tm[:, tt, :]
                nc.scalar.activation(
                    u, u, AF.Identity, scale=rs8b[:, tt:tt + 1],
                    bias=nmr8b[:, tt:tt + 1],
                )
                if ln2_affine:
                    nc.vector.tensor_tensor(u, u, lng_s, OP.mult)
                    nc.vector.tensor_tensor(u, u, lnb_s, OP.add)
                if b == nb - 1:
                    dma(out_dram[b][:, tt, :], u)
            if b < nb - 1:
                dma(out_dram[b], x_tm)

        emit_A_load(0)
        emit_A(0)
        emit_B1(0, dirs=(0,))
        emit_B3(0, dirs=(0,))
        emit_B1(0, dirs=(1,))
        emit_B3(0, dirs=(1,))
        emit_B2(0)
        if nb > 1:
            emit_A_load(1)
        emit_E(0)
        if nb > 2:
            emit_A_load(2)
        for k in range(nb):
            if k >= 1:
                emit_E(k)
            if k + 1 < nb:
                emit_A(k + 1)
            emit_E2(k)
            if k >= 1:
                emit_D_fin(k - 1)
            if k + 1 < nb:
                emit_B1(k + 1, dirs=(0,))
                emit_B3(k + 1, dirs=(0,))
                emit_B1(k + 1, dirs=(1,))
                emit_B3(k + 1, dirs=(1,))
                emit_B2(k + 1)
            emit_C(k)
            emit_D_proj(k)
            if k + 3 < nb:
                emit_A_load(k + 3)
        emit_D_fin(nb - 1)

    return nc


def kernel(**inputs):
    x = np.asarray(inputs["x"], np.float32)
    w = _host_weights(inputs)
    h = w.pop("host")

    ln2_affine = not (
        np.allclose(w["lng"], 1.0) and np.allclose(w["lnb"], 0.0)
    )
    nc = bacc.Bacc("TRN2", target_bir_lowering=False, debug=False)
    build(nc, nb=BL, ln2_affine=ln2_affine, es_struct=h["es_struct"],
          es_scale=h["es_scale"], s_xi=h["s_xi"], s_z=h["s_z"], s_g=h["s_g"])
    nc.compile()

    in_maps = []
    for c in range(NCORES):
        m = {"x": np.ascontiguousarray(x[c * BL:(c + 1) * BL])}
        m.update(w)
        in_maps.append(m)

    res = bass_utils.run_bass_kernel_spmd(nc, in_maps, core_ids=list(range(NCORES)))
    out = np.concatenate([res.results[c]["out"] for c in range(NCORES)], axis=0)
    return out.astype(np.float32)
